# revision 12
# baseline (speedup 1.0000x reference)
"""MetapathAttentionLayer Trainium2 kernel.

Math (per node n):
    scores[n, m] = sum_d x[m, n, d] * W[d, m]
    att = softmax(relu(scores), axis=m)      (8 metapaths)
    out[n, :] = elu(sum_m att[n, m] * x[m, n, :])

Strategy: shard nodes across 8 cores (data parallel). Per core, natural
layout [nodes(part), d(free)] in bf16:
  - scores: DVE tensor_tensor mul vs replicated-W tile, then sum over d
    as 3 bf16 tree-fold adds (128->16, 2x DVE mode) + one tensor_reduce
    (1 instr per group instead of a 32-op accum storm: DVE SEQ relief)
  - softmax: exp(relu(s)) == max(exp(s), 1); ACT Exp + DVE max/sum/recip
  - pooling: PE matmuls with diag(att_m) stationary (built by GPSIMD
    local_scatter, every DIAG_DVE_EVERY-th chunk on DVE to balance the
    two engines), accumulating over m into PSUM
  - elu(x) = max(x, exp(min(x, 0)) - 1): 2 ACT passes + 1 DVE
    scalar_tensor_tensor combine straight out of PSUM; bf16 output DMA
"""

import os
from contextlib import ExitStack

import numpy as np
import ml_dtypes

import concourse.bass as bass
import concourse.tile as tile
from concourse import bacc, mybir, library_config
import concourse.bass_utils as bass_utils

F32 = mybir.dt.float32
BF16 = mybir.dt.bfloat16
I16 = mybir.dt.int16
ALU = mybir.AluOpType
ACTF = mybir.ActivationFunctionType

NMETA = 8
N = 100000
D = 128
NCORES = 8
NC_RAW = N // NCORES          # 12500 nodes per core
CHUNK = 128                   # nodes per compute chunk (partition dim)
NC_PAD = 12544                # 98 chunks of 128
T_CHUNKS = 8                  # chunks per DMA T-tile (1024 nodes)
GROUP = 4                     # chunks per PSUM/elu group (psum bank = 512 f32)

# tunables
DIAG_DVE_EVERY = 3   # every k-th chunk builds diag via DVE tensor_scalar (0=off)


def kernel_body(tc, out_d, x_d, wb_d, sidx_d, icat_d,
                nc_pad=NC_PAD, t_chunks=T_CHUNKS, reps=1,
                diag_dve_every=DIAG_DVE_EVERY, comb_on_pool=False):
    nc = tc.nc
    with ExitStack() as ctx:
        const = ctx.enter_context(tc.tile_pool(name="const", bufs=1))
        xpool = ctx.enter_context(tc.tile_pool(name="x", bufs=3))
        opool = ctx.enter_context(tc.tile_pool(name="o", bufs=2))
        ppool = ctx.enter_context(tc.tile_pool(name="prod", bufs=3))
        fpool = ctx.enter_context(tc.tile_pool(name="fold", bufs=3))
        spool = ctx.enter_context(tc.tile_pool(name="smalls", bufs=6))
        dpool = ctx.enter_context(tc.tile_pool(name="diag", bufs=6))
        epool = ctx.enter_context(tc.tile_pool(name="elu", bufs=3))
        psum = ctx.enter_context(tc.tile_pool(name="ps", bufs=6, space="PSUM"))

        wb = const.tile([128, NMETA * D], BF16)
        nc.sync.dma_start(wb[:], wb_d[:])
        sidx = const.tile([128, NMETA], I16)
        nc.sync.dma_start(sidx[:], sidx_d[:])
        icat = const.tile([128, NMETA * D], BF16)
        nc.sync.dma_start(icat[:], icat_d[:])
        nc.gpsimd.load_library(library_config.local_scatter)

        chunk_idx = 0
        for _rep in range(reps):
            n0 = 0
            while n0 < nc_pad:
                ct = min(t_chunks, (nc_pad - n0) // CHUNK)
                nt = ct * CHUNK

                # node n = n0 + p*ct + c  ->  partition p, free chunk c
                X = xpool.tile([128, NMETA * nt], BF16, tag="X")
                for m in range(NMETA):
                    src = x_d[m, n0:n0 + nt, :].rearrange(
                        "(p c) d -> p (c d)", p=128)
                    nc.sync.dma_start(X[:, m * nt:(m + 1) * nt], src)
                Xv = X[:].rearrange("p (m c d) -> p m c d", m=NMETA, c=ct)

                out_sb = opool.tile([128, nt], BF16, tag="osb")

                for g0 in range(0, ct, GROUP):
                    gl = min(GROUP, ct - g0)
                    mc = NMETA * gl
                    ps = psum.tile([128, GROUP * D], F32, tag="ps")

                    # one batched multiply for the whole group of chunks
                    P = ppool.tile([128, NMETA * GROUP * D], BF16, tag="P")
                    Pv = P[:].rearrange("p (m c d) -> p m c d", m=NMETA, c=GROUP)
                    nc.vector.tensor_tensor(
                        out=Pv[:, :, :gl, :],
                        in0=Xv[:, :, g0:g0 + gl, :],
                        in1=wb[:].rearrange("p (m d) -> p m d", m=NMETA)
                              .unsqueeze(2).broadcast_to([128, NMETA, gl, D]),
                        op=ALU.mult,
                    )
                    # scores[p, (m c)] = sum_d P: fold d 128->16 in bf16 (2x
                    # DVE), then one 1x tensor_reduce.  P is laid out
                    # [p, m, c, d]; compact the c-dim when gl<GROUP.
                    Pg = Pv[:, :, :gl, :]
                    f1 = fpool.tile([128, NMETA * GROUP * D // 2], BF16, tag="f1")
                    f1v = f1[:].rearrange(
                        "p (m c d) -> p m c d", m=NMETA, c=GROUP)[:, :, :gl, :]
                    nc.vector.tensor_tensor(
                        out=f1v, in0=Pg[:, :, :, 0:64], in1=Pg[:, :, :, 64:128],
                        op=ALU.add)
                    f2 = fpool.tile([128, NMETA * GROUP * D // 4], BF16, tag="f2")
                    f2v = f2[:].rearrange(
                        "p (m c d) -> p m c d", m=NMETA, c=GROUP)[:, :, :gl, :]
                    nc.vector.tensor_tensor(
                        out=f2v, in0=f1v[:, :, :, 0:32], in1=f1v[:, :, :, 32:64],
                        op=ALU.add)
                    f3 = fpool.tile([128, NMETA * GROUP * D // 8], BF16, tag="f3")
                    f3v = f3[:].rearrange(
                        "p (m c d) -> p m c d", m=NMETA, c=GROUP)[:, :, :gl, :]
                    nc.vector.tensor_tensor(
                        out=f3v, in0=f2v[:, :, :, 0:16], in1=f2v[:, :, :, 16:32],
                        op=ALU.add)
                    scores = spool.tile([128, GROUP * NMETA], F32, tag="scores")
                    nc.vector.tensor_reduce(
                        out=scores[:, :mc].rearrange("p (m c) -> p m c", m=NMETA),
                        in_=f3v,
                        axis=mybir.AxisListType.X, op=ALU.add)

                    # softmax over m: att = e/sum(e), e = exp(relu(s)) = max(exp(s),1)
                    # scores layout is [p, (m c)]
                    e_raw = spool.tile([128, GROUP * NMETA], F32, tag="eraw")
                    nc.scalar.activation(
                        e_raw[:, :mc], scores[:, :mc], ACTF.Exp)
                    e_bf = spool.tile([128, GROUP * NMETA], BF16, tag="ebf")
                    nc.vector.tensor_scalar(
                        e_bf[:, :mc], e_raw[:, :mc],
                        1.0, None, ALU.max)
                    e_cm = e_bf[:, :mc].rearrange("p (m c) -> p c m", m=NMETA)
                    sums = spool.tile([128, GROUP], F32, tag="sums")
                    nc.vector.tensor_reduce(
                        out=sums[:, :gl], in_=e_cm,
                        axis=mybir.AxisListType.X,
                        op=ALU.add,
                    )
                    inv = spool.tile([128, GROUP], F32, tag="inv")
                    nc.vector.reciprocal(inv[:, :gl], sums[:, :gl])

                    for cg in range(gl):
                        c = g0 + cg
                        diag = dpool.tile([128, NMETA * D], BF16, tag="diag")
                        use_dve = (diag_dve_every and
                                   chunk_idx % diag_dve_every == 0)
                        if use_dve:
                            att_f = spool.tile([128, NMETA], F32, tag="attf")
                            nc.vector.tensor_scalar(
                                att_f[:], e_cm[:, cg, :],
                                inv[:, cg:cg + 1], None, ALU.mult)
                            for m in range(NMETA):
                                nc.vector.tensor_scalar(
                                    diag[:, m * D:(m + 1) * D],
                                    icat[:, m * D:(m + 1) * D],
                                    att_f[:, m:m + 1], None, ALU.mult)
                        else:
                            att = spool.tile([128, NMETA], BF16, tag="att")
                            nc.vector.tensor_scalar(
                                att[:], e_cm[:, cg, :],
                                inv[:, cg:cg + 1], None, ALU.mult)
                            nc.gpsimd.local_scatter(
                                diag[:], att[:], sidx[:],
                                channels=128, num_elems=NMETA * D,
                                num_idxs=NMETA)
                        for m in range(NMETA):
                            nc.tensor.matmul(
                                out=ps[:, cg * D:(cg + 1) * D],
                                lhsT=diag[:, m * D:(m + 1) * D],
                                rhs=Xv[:, m, c, :],
                                start=(m == 0),
                                stop=(m == NMETA - 1),
                            )
                        chunk_idx += 1

                    # elu(x) = max(x, exp(min(x,0)) - 1)
                    w = gl * D
                    t = epool.tile([128, GROUP * D], F32, tag="t")
                    nc.scalar.activation(t[:, :w], ps[:, :w], ACTF.Relu,
                                         scale=-1.0)
                    e2 = epool.tile([128, GROUP * D], F32, tag="e2")
                    nc.scalar.activation(e2[:, :w], t[:, :w], ACTF.Exp,
                                         scale=-1.0)
                    # out = max(ps, e2 - 1) in one fused op
                    eng = nc.gpsimd if comb_on_pool else nc.vector
                    eng.scalar_tensor_tensor(
                        out=out_sb[:, g0 * D:g0 * D + w],
                        in0=e2[:, :w], scalar=-1.0, in1=ps[:, :w],
                        op0=ALU.add, op1=ALU.max)

                dsto = out_d[n0:n0 + nt, :].rearrange("(p c) d -> p (c d)", p=128)
                nc.sync.dma_start(dsto, out_sb[:])
                n0 += nt


def host_inputs(x_np, w_np, nc_pad=NC_PAD):
    """Build per-core input maps from full fp32 inputs."""
    in_maps = []
    wbig = np.ascontiguousarray(
        np.broadcast_to(w_np.T.reshape(1, NMETA * D), (128, NMETA * D))
    ).astype(ml_dtypes.bfloat16)
    sidx = (np.arange(NMETA)[None, :] * D + np.arange(128)[:, None]).astype(np.int16)
    icat = np.ascontiguousarray(
        np.tile(np.eye(128, dtype=np.float32), (1, NMETA))
    ).astype(ml_dtypes.bfloat16)
    nc_raw = x_np.shape[1] // NCORES
    for c in range(NCORES):
        xs = x_np[:, c * nc_raw:(c + 1) * nc_raw, :]
        xp = np.zeros((NMETA, nc_pad, D), dtype=ml_dtypes.bfloat16)
        xp[:, :nc_raw, :] = xs.astype(ml_dtypes.bfloat16)
        in_maps.append({"x": xp, "wb": wbig, "sidx": sidx, "icat": icat})
    return in_maps


_CACHE = {}


def build(reps=1, **kw):
    key = (reps, tuple(sorted(kw.items())))
    if key in _CACHE:
        return _CACHE[key]
    nc = bacc.Bacc("TRN2", target_bir_lowering=False, debug=False,
                   num_devices=NCORES)
    x = nc.dram_tensor("x", [NMETA, NC_PAD, D], BF16, kind="ExternalInput").ap()
    wb = nc.dram_tensor("wb", [128, NMETA * D], BF16, kind="ExternalInput").ap()
    sidx = nc.dram_tensor("sidx", [128, NMETA], I16, kind="ExternalInput").ap()
    icat = nc.dram_tensor("icat", [128, NMETA * D], BF16, kind="ExternalInput").ap()
    out = nc.dram_tensor("out", [NC_PAD, D], BF16, kind="ExternalOutput").ap()
    with tile.TileContext(nc) as tc:
        kernel_body(tc, out, x, wb, sidx, icat, reps=reps, **kw)
    nc.compile()
    _CACHE[key] = nc
    return nc


def run(input, W, trace=False, **trace_kwargs):
    x_np = np.asarray(input, dtype=np.float32)
    w_np = np.asarray(W, dtype=np.float32)
    nc = build()
    in_maps = host_inputs(x_np, w_np)
    res = bass_utils.run_bass_kernel_spmd(
        nc, in_maps, core_ids=list(range(NCORES)), trace=trace, **trace_kwargs)
    nc_raw = x_np.shape[1] // NCORES
    full = np.concatenate(
        [np.asarray(res.results[c]["out"][:nc_raw], dtype=np.float32)
         for c in range(NCORES)], axis=0)
    return full, res


def kernel(input, W):
    out, _ = run(input, W, trace=False)
    return out


# ---------------------------------------------------------------------------
# Timing harness (test-only): persistent jit over the bass_exec primitive so
# repeated executions reuse device-resident inputs. HW kernel time is derived
# from the slope between an R-repeat NEFF and the 1-repeat NEFF.
# ---------------------------------------------------------------------------

def make_runner(nc):
    import jax
    from jax.experimental.shard_map import shard_map
    from jax.sharding import Mesh, PartitionSpec, NamedSharding
    from concourse import bass2jax as b2j

    b2j.install_neuronx_cc_hook()
    partition_name = nc.partition_id_tensor.name if nc.partition_id_tensor else None
    in_names, out_names, out_avals, zero_outs = [], [], [], []
    for alloc in nc.m.functions[0].allocations:
        if not isinstance(alloc, mybir.MemoryLocationSet):
            continue
        name = alloc.memorylocations[0].name
        if alloc.kind == "ExternalInput":
            if name != partition_name:
                in_names.append(name)
        elif alloc.kind == "ExternalOutput":
            out_names.append(name)
            shape = tuple(alloc.tensor_shape)
            dtype = mybir.dt.np(alloc.dtype)
            out_avals.append(jax.core.ShapedArray(shape, dtype))
            zero_outs.append(np.zeros(shape, dtype))
    n_params = len(in_names)
    n_outs = len(out_avals)
    all_names = in_names + out_names + ([partition_name] if partition_name else [])

    def _body(*args):
        operands = list(args)
        if partition_name is not None:
            operands.append(b2j.partition_id_tensor())
        outs = b2j._bass_exec_p.bind(
            *operands,
            out_avals=tuple(out_avals),
            in_names=tuple(all_names),
            out_names=tuple(out_names),
            lowering_input_output_aliases=(),
            sim_require_finite=True,
            sim_require_nnan=True,
            nc=nc,
        )
        return tuple(outs)

    devices = jax.devices()[:NCORES]
    mesh = Mesh(np.asarray(devices), ("core",))
    in_specs = (PartitionSpec("core"),) * (n_params + n_outs)
    out_specs = (PartitionSpec("core"),) * n_outs
    donate = tuple(range(n_params, n_params + n_outs))
    sharded = jax.jit(
        shard_map(_body, mesh=mesh, in_specs=in_specs, out_specs=out_specs,
                  check_rep=False),
        donate_argnums=donate, keep_unused=True)
    sharding = NamedSharding(mesh, PartitionSpec("core"))
    return sharded, in_names, zero_outs, sharding


class _TimedRunner:
    def __init__(self, nc, in_maps):
        import jax
        self.jax = jax
        sharded, in_names, zero_outs, sharding = make_runner(nc)
        self.sharded = sharded
        concat_in = [
            np.concatenate([in_maps[c][n] for c in range(NCORES)], axis=0)
            for n in in_names
        ]
        self.xs = [jax.device_put(a, sharding) for a in concat_in]
        self.zero_outs = zero_outs
        self.sharding = sharding

    def _zset(self):
        return [
            self.jax.device_put(
                np.zeros((NCORES * z.shape[0], *z.shape[1:]), z.dtype),
                self.sharding)
            for z in self.zero_outs
        ]

    def piped(self, reps):
        import time as _t
        zsets = [self._zset() for _ in range(reps + 1)]
        self.jax.block_until_ready(zsets)
        self.jax.block_until_ready(self.xs)
        o = self.sharded(*self.xs, *zsets[0])
        self.jax.block_until_ready(o)
        _ = self.jax.device_get(o[0])
        t0 = _t.perf_counter()
        outs = [self.sharded(*self.xs, *zsets[1 + k]) for k in range(reps)]
        self.jax.block_until_ready(outs)
        # force true device completion: fetch the last output's bytes
        _ = self.jax.device_get(outs[-1][0])
        return (_t.perf_counter() - t0) / reps


def measure(input, W, reps=12, neff_reps=9, rounds=4, **kw):
    """Estimate per-iteration HW time via multi-repeat NEFF slope.

    Interleaves rounds of (1-repeat NEFF, R-repeat NEFF) piped timings and
    takes the min across rounds for each to reject dispatch-overhead noise.
    """
    x_np = np.asarray(input, dtype=np.float32)
    w_np = np.asarray(W, dtype=np.float32)
    in_maps = host_inputs(x_np, w_np)

    nc1 = build(reps=1, **kw)
    ncr = build(reps=neff_reps, **kw)
    r1 = _TimedRunner(nc1, in_maps)
    rr = _TimedRunner(ncr, in_maps)
    t1s, trs = [], []
    for _ in range(rounds):
        t1s.append(r1.piped(reps))
        trs.append(rr.piped(reps))
    t1, tr = min(t1s), min(trs)
    slope = (tr - t1) / (neff_reps - 1)
    return t1, tr, slope, t1s, trs



# revision 18
# speedup vs baseline: 1.0345x; 1.0345x over previous
"""MetapathAttentionLayer Trainium2 kernel.

Math (per node n):
    scores[n, m] = sum_d x[m, n, d] * W[d, m]
    att = softmax(relu(scores), axis=m)      (8 metapaths)
    out[n, :] = elu(sum_m att[n, m] * x[m, n, :])

Strategy: shard nodes across 8 cores (data parallel). Per core, natural
layout [nodes(part), d(free)] in bf16:
  - scores: DVE tensor_tensor mul vs replicated-W tile, then sum over d
    as 3 bf16 tree-fold adds (128->16, 2x DVE mode) + one tensor_reduce
    (1 instr per group instead of a 32-op accum storm: DVE SEQ relief)
  - softmax: exp(relu(s)) == max(exp(s), 1); ACT Exp + DVE max/sum/recip
  - pooling: PE matmuls with diag(att_m) stationary (built by GPSIMD
    local_scatter, every DIAG_DVE_EVERY-th chunk on DVE to balance the
    two engines), accumulating over m into PSUM
  - elu(x) = max(x, exp(min(x, 0)) - 1): 2 ACT passes + 1 DVE
    scalar_tensor_tensor combine straight out of PSUM; bf16 output DMA
"""

import os
from contextlib import ExitStack

import numpy as np
import ml_dtypes

import concourse.bass as bass
import concourse.tile as tile
from concourse import bacc, mybir, library_config
import concourse.bass_utils as bass_utils

F32 = mybir.dt.float32
BF16 = mybir.dt.bfloat16
I16 = mybir.dt.int16
ALU = mybir.AluOpType
ACTF = mybir.ActivationFunctionType

NMETA = 8
N = 100000
D = 128
NCORES = 8
NC_RAW = N // NCORES          # 12500 nodes per core
CHUNK = 128                   # nodes per compute chunk (partition dim)
NC_PAD = 12544                # 98 chunks of 128
T_CHUNKS = 8                  # chunks per DMA T-tile (1024 nodes)
GROUP = 4                     # chunks per PSUM/elu group (psum bank = 512 f32)

# tunables
DIAG_DVE_EVERY = 3   # every k-th chunk builds diag via DVE tensor_scalar (0=off)
K_PE = 4             # metapaths whose scores run on the PE (d-major path)


def kernel_body(tc, out_d, x_d, wb_d, sidx_d, icat_d, wm_d,
                nc_pad=NC_PAD, t_chunks=T_CHUNKS, reps=1,
                diag_dve_every=DIAG_DVE_EVERY, k_pe=K_PE, comb_on_pool=False):
    nc = tc.nc
    md = NMETA - k_pe             # metapaths on the DVE score path
    with ExitStack() as ctx:
        const = ctx.enter_context(tc.tile_pool(name="const", bufs=1))
        xpool = ctx.enter_context(tc.tile_pool(name="x", bufs=3))
        xtpool = ctx.enter_context(tc.tile_pool(name="xt", bufs=3))
        stpool = ctx.enter_context(tc.tile_pool(name="st", bufs=3))
        opool = ctx.enter_context(tc.tile_pool(name="o", bufs=2))
        ppool = ctx.enter_context(tc.tile_pool(name="prod", bufs=3))
        fpool = ctx.enter_context(tc.tile_pool(name="fold", bufs=3))
        spool = ctx.enter_context(tc.tile_pool(name="smalls", bufs=6))
        dpool = ctx.enter_context(tc.tile_pool(name="diag", bufs=6))
        epool = ctx.enter_context(tc.tile_pool(name="elu", bufs=3))
        psum = ctx.enter_context(tc.tile_pool(name="ps", bufs=4, space="PSUM"))
        sscp = ctx.enter_context(tc.tile_pool(name="ssc", bufs=2, space="PSUM"))
        pscp = ctx.enter_context(tc.tile_pool(name="psc", bufs=2, space="PSUM"))

        wb = const.tile([128, NMETA * D], BF16)
        nc.sync.dma_start(wb[:], wb_d[:])
        sidx = const.tile([128, NMETA], I16)
        nc.sync.dma_start(sidx[:], sidx_d[:])
        icat = const.tile([128, NMETA * D], BF16)
        nc.sync.dma_start(icat[:], icat_d[:])
        wmat = const.tile([128, NMETA * NMETA], BF16)
        nc.sync.dma_start(wmat[:], wm_d[:])
        nc.gpsimd.load_library(library_config.local_scatter)

        chunk_idx = 0
        for _rep in range(reps):
            n0 = 0
            while n0 < nc_pad:
                ct = min(t_chunks, (nc_pad - n0) // CHUNK)
                nt = ct * CHUNK

                # node n = n0 + p*ct + c  ->  partition p, free chunk c
                X = xpool.tile([128, NMETA * nt], BF16, tag="X")
                for m in range(NMETA):
                    src = x_d[m, n0:n0 + nt, :].rearrange(
                        "(p c) d -> p (c d)", p=128)
                    nc.sync.dma_start(X[:, m * nt:(m + 1) * nt], src)
                Xv = X[:].rearrange("p (m c d) -> p m c d", m=NMETA, c=ct)

                # PE score path for m < k_pe: XBAR-transposed load of the
                # same slabs as [d(part), node(free)], then scores via
                # 1-column-W matmuls into S^T psum rows, bf16 copy to SBUF.
                if k_pe:
                    XT = xtpool.tile([128, k_pe * nt], BF16, tag="XT")
                    for m in range(k_pe):
                        nc.sync.dma_start_transpose(
                            XT[:, m * nt:(m + 1) * nt],
                            x_d[m, n0:n0 + nt, :])
                    STb = stpool.tile([k_pe, nt], BF16, tag="STb")
                    for b0 in range(0, nt, 512):
                        bl = min(512, nt - b0)
                        ssc = sscp.tile([k_pe, 512], F32, tag="ssc")
                        # accumulate k masked-W matmuls: row m comes from
                        # slab m only (other columns of wmat block m are 0)
                        for m in range(k_pe):
                            nc.tensor.matmul(
                                out=ssc[:, :bl],
                                lhsT=wmat[:].rearrange(
                                    "p (m k) -> p m k", m=NMETA)[:, m, 0:k_pe],
                                rhs=XT[:, m * nt + b0:m * nt + b0 + bl],
                                start=(m == 0), stop=(m == k_pe - 1))
                        nc.scalar.activation(
                            STb[:, b0:b0 + bl], ssc[:, :bl], ACTF.Copy)
                    # S^T[:, (p ct + c)] view for per-chunk re-blocking
                    STv = STb[:].rearrange("q (p c) -> q c p", c=ct)

                out_sb = opool.tile([128, nt], BF16, tag="osb")

                for g0 in range(0, ct, GROUP):
                    gl = min(GROUP, ct - g0)
                    mc = md * gl
                    ps = psum.tile([128, GROUP * D], F32, tag="ps")

                    # re-block PE scores to node-major via tiny identity
                    # matmuls: out[p, m] = S^T[m, p*ct + c]
                    if k_pe:
                        ps_sc = pscp.tile([128, GROUP * 16], F32, tag="psc")
                        for cg in range(gl):
                            nc.tensor.matmul(
                                out=ps_sc[:, cg * 16:(cg + 1) * 16],
                                lhsT=STv[:, g0 + cg, :],
                                rhs=icat[0:k_pe, 0:16],
                                start=True, stop=True)

                    # DVE score path for m >= k_pe
                    P = ppool.tile([128, NMETA * GROUP * D], BF16, tag="P")
                    Pv = P[:, :md * GROUP * D].rearrange(
                        "p (m c d) -> p m c d", m=md, c=GROUP)
                    nc.vector.tensor_tensor(
                        out=Pv[:, :, :gl, :],
                        in0=Xv[:, k_pe:, g0:g0 + gl, :],
                        in1=wb[:].rearrange("p (m d) -> p m d", m=NMETA)
                              [:, k_pe:, :]
                              .unsqueeze(2).broadcast_to([128, md, gl, D]),
                        op=ALU.mult,
                    )
                    # scores[p, (m c)] = sum_d P: fold d 128->16 in bf16 (2x
                    # DVE), then one 1x tensor_reduce.
                    Pg = Pv[:, :, :gl, :]
                    f1 = fpool.tile([128, NMETA * GROUP * D // 2], BF16, tag="f1")
                    f1v = f1[:, :md * GROUP * D // 2].rearrange(
                        "p (m c d) -> p m c d", m=md, c=GROUP)[:, :, :gl, :]
                    nc.vector.tensor_tensor(
                        out=f1v, in0=Pg[:, :, :, 0:64], in1=Pg[:, :, :, 64:128],
                        op=ALU.add)
                    f2 = fpool.tile([128, NMETA * GROUP * D // 4], BF16, tag="f2")
                    f2v = f2[:, :md * GROUP * D // 4].rearrange(
                        "p (m c d) -> p m c d", m=md, c=GROUP)[:, :, :gl, :]
                    nc.vector.tensor_tensor(
                        out=f2v, in0=f1v[:, :, :, 0:32], in1=f1v[:, :, :, 32:64],
                        op=ALU.add)
                    f3 = fpool.tile([128, NMETA * GROUP * D // 8], BF16, tag="f3")
                    f3v = f3[:, :md * GROUP * D // 8].rearrange(
                        "p (m c d) -> p m c d", m=md, c=GROUP)[:, :, :gl, :]
                    nc.vector.tensor_tensor(
                        out=f3v, in0=f2v[:, :, :, 0:16], in1=f2v[:, :, :, 16:32],
                        op=ALU.add)
                    scores = spool.tile([128, GROUP * NMETA], F32, tag="scores")
                    nc.vector.tensor_reduce(
                        out=scores[:, :mc].rearrange("p (m c) -> p m c", m=md),
                        in_=f3v,
                        axis=mybir.AxisListType.X, op=ALU.add)

                    # softmax over m: merge both score pieces into one
                    # (c m)-major e tile: e = exp(relu(s)) = max(exp(s), 1)
                    e_bf = spool.tile([128, GROUP * NMETA], BF16, tag="ebf")
                    e_cm = e_bf[:, :gl * NMETA].rearrange(
                        "p (c m) -> p c m", m=NMETA)
                    if k_pe:
                        nc.scalar.activation(
                            e_cm[:, :, 0:k_pe],
                            ps_sc[:, :gl * 16].rearrange(
                                "p (c s) -> p c s", s=16)[:, :, 0:k_pe],
                            ACTF.Exp)
                    nc.scalar.activation(
                        e_cm[:, :, k_pe:].rearrange("p c m -> p m c"),
                        scores[:, :mc].rearrange("p (m c) -> p m c", m=md),
                        ACTF.Exp)
                    e_max = spool.tile([128, GROUP * NMETA], BF16, tag="emax")
                    nc.vector.tensor_scalar(
                        e_max[:, :gl * NMETA], e_bf[:, :gl * NMETA],
                        1.0, None, ALU.max)
                    em_cm = e_max[:, :gl * NMETA].rearrange(
                        "p (c m) -> p c m", m=NMETA)
                    sums = spool.tile([128, GROUP], F32, tag="sums")
                    nc.vector.tensor_reduce(
                        out=sums[:, :gl], in_=em_cm,
                        axis=mybir.AxisListType.X,
                        op=ALU.add,
                    )
                    inv = spool.tile([128, GROUP], F32, tag="inv")
                    nc.vector.reciprocal(inv[:, :gl], sums[:, :gl])

                    for cg in range(gl):
                        c = g0 + cg
                        diag = dpool.tile([128, NMETA * D], BF16, tag="diag")
                        use_dve = (diag_dve_every and
                                   chunk_idx % diag_dve_every == 0)
                        if use_dve:
                            att_f = spool.tile([128, NMETA], F32, tag="attf")
                            nc.vector.tensor_scalar(
                                att_f[:], em_cm[:, cg, :],
                                inv[:, cg:cg + 1], None, ALU.mult)
                            for m in range(NMETA):
                                nc.vector.tensor_scalar(
                                    diag[:, m * D:(m + 1) * D],
                                    icat[:, m * D:(m + 1) * D],
                                    att_f[:, m:m + 1], None, ALU.mult)
                        else:
                            att = spool.tile([128, NMETA], BF16, tag="att")
                            nc.vector.tensor_scalar(
                                att[:], em_cm[:, cg, :],
                                inv[:, cg:cg + 1], None, ALU.mult)
                            nc.gpsimd.local_scatter(
                                diag[:], att[:], sidx[:],
                                channels=128, num_elems=NMETA * D,
                                num_idxs=NMETA)
                        for m in range(NMETA):
                            nc.tensor.matmul(
                                out=ps[:, cg * D:(cg + 1) * D],
                                lhsT=diag[:, m * D:(m + 1) * D],
                                rhs=Xv[:, m, c, :],
                                start=(m == 0),
                                stop=(m == NMETA - 1),
                            )
                        chunk_idx += 1

                    # elu(x) = max(x, exp(min(x,0)) - 1)
                    w = gl * D
                    t = epool.tile([128, GROUP * D], F32, tag="t")
                    nc.scalar.activation(t[:, :w], ps[:, :w], ACTF.Relu,
                                         scale=-1.0)
                    e2 = epool.tile([128, GROUP * D], F32, tag="e2")
                    nc.scalar.activation(e2[:, :w], t[:, :w], ACTF.Exp,
                                         scale=-1.0)
                    # out = max(ps, e2 - 1) in one fused op
                    eng = nc.gpsimd if comb_on_pool else nc.vector
                    eng.scalar_tensor_tensor(
                        out=out_sb[:, g0 * D:g0 * D + w],
                        in0=e2[:, :w], scalar=-1.0, in1=ps[:, :w],
                        op0=ALU.add, op1=ALU.max)

                dsto = out_d[n0:n0 + nt, :].rearrange("(p c) d -> p (c d)", p=128)
                nc.sync.dma_start(dsto, out_sb[:])
                n0 += nt


def host_inputs(x_np, w_np, nc_pad=NC_PAD):
    """Build per-core input maps from full fp32 inputs."""
    in_maps = []
    wbig = np.ascontiguousarray(
        np.broadcast_to(w_np.T.reshape(1, NMETA * D), (128, NMETA * D))
    ).astype(ml_dtypes.bfloat16)
    sidx = (np.arange(NMETA)[None, :] * D + np.arange(128)[:, None]).astype(np.int16)
    icat = np.ascontiguousarray(
        np.tile(np.eye(128, dtype=np.float32), (1, NMETA))
    ).astype(ml_dtypes.bfloat16)
    # masked-W blocks: wmk[:, m*NMETA + m'] = W[:, m] iff m' == m else 0
    wmk_f = np.zeros((128, NMETA * NMETA), dtype=np.float32)
    for m in range(NMETA):
        wmk_f[:, m * NMETA + m] = w_np[:, m]
    wmk = wmk_f.astype(ml_dtypes.bfloat16)
    nc_raw = x_np.shape[1] // NCORES
    for c in range(NCORES):
        xs = x_np[:, c * nc_raw:(c + 1) * nc_raw, :]
        xp = np.zeros((NMETA, nc_pad, D), dtype=ml_dtypes.bfloat16)
        xp[:, :nc_raw, :] = xs.astype(ml_dtypes.bfloat16)
        in_maps.append({"x": xp, "wb": wbig, "sidx": sidx, "icat": icat,
                        "wm": wmk})
    return in_maps


_CACHE = {}


def build(reps=1, **kw):
    key = (reps, tuple(sorted(kw.items())))
    if key in _CACHE:
        return _CACHE[key]
    nc = bacc.Bacc("TRN2", target_bir_lowering=False, debug=False,
                   num_devices=NCORES)
    x = nc.dram_tensor("x", [NMETA, NC_PAD, D], BF16, kind="ExternalInput").ap()
    wb = nc.dram_tensor("wb", [128, NMETA * D], BF16, kind="ExternalInput").ap()
    sidx = nc.dram_tensor("sidx", [128, NMETA], I16, kind="ExternalInput").ap()
    icat = nc.dram_tensor("icat", [128, NMETA * D], BF16, kind="ExternalInput").ap()
    wm = nc.dram_tensor("wm", [128, NMETA * NMETA], BF16,
                        kind="ExternalInput").ap()
    out = nc.dram_tensor("out", [NC_PAD, D], BF16, kind="ExternalOutput").ap()
    with tile.TileContext(nc) as tc:
        kernel_body(tc, out, x, wb, sidx, icat, wm, reps=reps, **kw)
    nc.compile()
    _CACHE[key] = nc
    return nc


def run(input, W, trace=False, **trace_kwargs):
    x_np = np.asarray(input, dtype=np.float32)
    w_np = np.asarray(W, dtype=np.float32)
    nc = build()
    in_maps = host_inputs(x_np, w_np)
    res = bass_utils.run_bass_kernel_spmd(
        nc, in_maps, core_ids=list(range(NCORES)), trace=trace, **trace_kwargs)
    nc_raw = x_np.shape[1] // NCORES
    full = np.concatenate(
        [np.asarray(res.results[c]["out"][:nc_raw], dtype=np.float32)
         for c in range(NCORES)], axis=0)
    return full, res


def kernel(input, W):
    out, _ = run(input, W, trace=False)
    return out


# ---------------------------------------------------------------------------
# Timing harness (test-only): persistent jit over the bass_exec primitive so
# repeated executions reuse device-resident inputs. HW kernel time is derived
# from the slope between an R-repeat NEFF and the 1-repeat NEFF.
# ---------------------------------------------------------------------------

def make_runner(nc):
    import jax
    from jax.experimental.shard_map import shard_map
    from jax.sharding import Mesh, PartitionSpec, NamedSharding
    from concourse import bass2jax as b2j

    b2j.install_neuronx_cc_hook()
    partition_name = nc.partition_id_tensor.name if nc.partition_id_tensor else None
    in_names, out_names, out_avals, zero_outs = [], [], [], []
    for alloc in nc.m.functions[0].allocations:
        if not isinstance(alloc, mybir.MemoryLocationSet):
            continue
        name = alloc.memorylocations[0].name
        if alloc.kind == "ExternalInput":
            if name != partition_name:
                in_names.append(name)
        elif alloc.kind == "ExternalOutput":
            out_names.append(name)
            shape = tuple(alloc.tensor_shape)
            dtype = mybir.dt.np(alloc.dtype)
            out_avals.append(jax.core.ShapedArray(shape, dtype))
            zero_outs.append(np.zeros(shape, dtype))
    n_params = len(in_names)
    n_outs = len(out_avals)
    all_names = in_names + out_names + ([partition_name] if partition_name else [])

    def _body(*args):
        operands = list(args)
        if partition_name is not None:
            operands.append(b2j.partition_id_tensor())
        outs = b2j._bass_exec_p.bind(
            *operands,
            out_avals=tuple(out_avals),
            in_names=tuple(all_names),
            out_names=tuple(out_names),
            lowering_input_output_aliases=(),
            sim_require_finite=True,
            sim_require_nnan=True,
            nc=nc,
        )
        return tuple(outs)

    devices = jax.devices()[:NCORES]
    mesh = Mesh(np.asarray(devices), ("core",))
    in_specs = (PartitionSpec("core"),) * (n_params + n_outs)
    out_specs = (PartitionSpec("core"),) * n_outs
    donate = tuple(range(n_params, n_params + n_outs))
    sharded = jax.jit(
        shard_map(_body, mesh=mesh, in_specs=in_specs, out_specs=out_specs,
                  check_rep=False),
        donate_argnums=donate, keep_unused=True)
    sharding = NamedSharding(mesh, PartitionSpec("core"))
    return sharded, in_names, zero_outs, sharding


class _TimedRunner:
    def __init__(self, nc, in_maps):
        import jax
        self.jax = jax
        sharded, in_names, zero_outs, sharding = make_runner(nc)
        self.sharded = sharded
        concat_in = [
            np.concatenate([in_maps[c][n] for c in range(NCORES)], axis=0)
            for n in in_names
        ]
        self.xs = [jax.device_put(a, sharding) for a in concat_in]
        self.zero_outs = zero_outs
        self.sharding = sharding

    def _zset(self):
        return [
            self.jax.device_put(
                np.zeros((NCORES * z.shape[0], *z.shape[1:]), z.dtype),
                self.sharding)
            for z in self.zero_outs
        ]

    def piped(self, reps):
        import time as _t
        zsets = [self._zset() for _ in range(reps + 1)]
        self.jax.block_until_ready(zsets)
        self.jax.block_until_ready(self.xs)
        o = self.sharded(*self.xs, *zsets[0])
        self.jax.block_until_ready(o)
        _ = self.jax.device_get(o[0])
        t0 = _t.perf_counter()
        outs = [self.sharded(*self.xs, *zsets[1 + k]) for k in range(reps)]
        self.jax.block_until_ready(outs)
        # force true device completion: fetch the last output's bytes
        _ = self.jax.device_get(outs[-1][0])
        return (_t.perf_counter() - t0) / reps


def measure(input, W, reps=12, neff_reps=9, rounds=4, **kw):
    """Estimate per-iteration HW time via multi-repeat NEFF slope.

    Interleaves rounds of (1-repeat NEFF, R-repeat NEFF) piped timings and
    takes the min across rounds for each to reject dispatch-overhead noise.
    """
    x_np = np.asarray(input, dtype=np.float32)
    w_np = np.asarray(W, dtype=np.float32)
    in_maps = host_inputs(x_np, w_np)

    nc1 = build(reps=1, **kw)
    ncr = build(reps=neff_reps, **kw)
    r1 = _TimedRunner(nc1, in_maps)
    rr = _TimedRunner(ncr, in_maps)
    t1s, trs = [], []
    for _ in range(rounds):
        t1s.append(r1.piped(reps))
        trs.append(rr.piped(reps))
    t1, tr = min(t1s), min(trs)
    slope = (tr - t1) / (neff_reps - 1)
    return t1, tr, slope, t1s, trs



# revision 20
# speedup vs baseline: 1.0518x; 1.0167x over previous
"""MetapathAttentionLayer Trainium2 kernel.

Math (per node n):
    scores[n, m] = sum_d x[m, n, d] * W[d, m]
    att = softmax(relu(scores), axis=m)      (8 metapaths)
    out[n, :] = elu(sum_m att[n, m] * x[m, n, :])

Strategy: shard nodes across 8 cores (data parallel). Per core, natural
layout [nodes(part), d(free)] in bf16:
  - scores: DVE tensor_tensor mul vs replicated-W tile, then sum over d
    as 3 bf16 tree-fold adds (128->16, 2x DVE mode) + one tensor_reduce
    (1 instr per group instead of a 32-op accum storm: DVE SEQ relief)
  - softmax: exp(relu(s)) == max(exp(s), 1); ACT Exp + DVE max/sum/recip
  - pooling: PE matmuls with diag(att_m) stationary (built by GPSIMD
    local_scatter, every DIAG_DVE_EVERY-th chunk on DVE to balance the
    two engines), accumulating over m into PSUM
  - elu(x) = max(x, exp(min(x, 0)) - 1): 2 ACT passes + 1 DVE
    scalar_tensor_tensor combine straight out of PSUM; bf16 output DMA
"""

import os
from contextlib import ExitStack

import numpy as np
import ml_dtypes

import concourse.bass as bass
import concourse.tile as tile
from concourse import bacc, mybir, library_config
import concourse.bass_utils as bass_utils

F32 = mybir.dt.float32
BF16 = mybir.dt.bfloat16
I16 = mybir.dt.int16
ALU = mybir.AluOpType
ACTF = mybir.ActivationFunctionType

NMETA = 8
N = 100000
D = 128
NCORES = 8
NC_RAW = N // NCORES          # 12500 nodes per core
CHUNK = 128                   # nodes per compute chunk (partition dim)
NC_PAD = 12544                # 98 chunks of 128
T_CHUNKS = 8                  # chunks per DMA T-tile (1024 nodes)
GROUP = 4                     # chunks per PSUM/elu group (psum bank = 512 f32)

# tunables
DIAG_DVE_EVERY = 3   # every k-th chunk builds diag via DVE tensor_scalar (0=off)
K_PE = 4             # metapaths whose scores run on the PE (d-major path)


def kernel_body(tc, out_d, x_d, wb_d, sidx_d, icat_d, wm_d, xt_d,
                nc_pad=NC_PAD, t_chunks=T_CHUNKS, reps=1,
                diag_dve_every=DIAG_DVE_EVERY, k_pe=K_PE, comb_on_pool=False):
    nc = tc.nc
    md = NMETA - k_pe             # metapaths on the DVE score path
    with ExitStack() as ctx:
        const = ctx.enter_context(tc.tile_pool(name="const", bufs=1))
        xpool = ctx.enter_context(tc.tile_pool(name="x", bufs=3))
        xtpool = ctx.enter_context(tc.tile_pool(name="xt", bufs=3))
        stpool = ctx.enter_context(tc.tile_pool(name="st", bufs=3))
        opool = ctx.enter_context(tc.tile_pool(name="o", bufs=2))
        ppool = ctx.enter_context(tc.tile_pool(name="prod", bufs=3))
        fpool = ctx.enter_context(tc.tile_pool(name="fold", bufs=3))
        spool = ctx.enter_context(tc.tile_pool(name="smalls", bufs=6))
        dpool = ctx.enter_context(tc.tile_pool(name="diag", bufs=6))
        epool = ctx.enter_context(tc.tile_pool(name="elu", bufs=3))
        psum = ctx.enter_context(tc.tile_pool(name="ps", bufs=4, space="PSUM"))
        sscp = ctx.enter_context(tc.tile_pool(name="ssc", bufs=2, space="PSUM"))
        pscp = ctx.enter_context(tc.tile_pool(name="psc", bufs=2, space="PSUM"))

        wb = const.tile([128, NMETA * D], BF16)
        nc.sync.dma_start(wb[:], wb_d[:])
        sidx = const.tile([128, NMETA], I16)
        nc.sync.dma_start(sidx[:], sidx_d[:])
        icat = const.tile([128, NMETA * D], BF16)
        nc.sync.dma_start(icat[:], icat_d[:])
        wmat = const.tile([128, NMETA * NMETA], BF16)
        nc.sync.dma_start(wmat[:], wm_d[:])
        nc.gpsimd.load_library(library_config.local_scatter)

        chunk_idx = 0
        for _rep in range(reps):
            n0 = 0
            while n0 < nc_pad:
                ct = min(t_chunks, (nc_pad - n0) // CHUNK)
                nt = ct * CHUNK

                # node n = n0 + p*ct + c  ->  partition p, free chunk c
                X = xpool.tile([128, NMETA * nt], BF16, tag="X")
                nc.sync.dma_start(
                    X[:].rearrange("p (m c d) -> p m c d", m=NMETA, c=ct),
                    x_d[:, n0:n0 + nt, :].rearrange(
                        "m (p c) d -> p m c d", p=128))
                Xv = X[:].rearrange("p (m c d) -> p m c d", m=NMETA, c=ct)

                # PE score path for m < k_pe: XBAR-transposed load of the
                # same slabs as [d(part), node(free)], then scores via
                # 1-column-W matmuls into S^T psum rows, bf16 copy to SBUF.
                if k_pe:
                    XT = xtpool.tile([128, k_pe * nt], BF16, tag="XT")
                    nc.sync.dma_start_transpose(
                        XT[:], xt_d[k_pe * n0:k_pe * (n0 + nt), :])
                    STb = stpool.tile([k_pe, nt], BF16, tag="STb")
                    for b0 in range(0, nt, 512):
                        bl = min(512, nt - b0)
                        ssc = sscp.tile([k_pe, 512], F32, tag="ssc")
                        # accumulate k masked-W matmuls: row m comes from
                        # slab m only (other columns of wmat block m are 0)
                        for m in range(k_pe):
                            nc.tensor.matmul(
                                out=ssc[:, :bl],
                                lhsT=wmat[:].rearrange(
                                    "p (m k) -> p m k", m=NMETA)[:, m, 0:k_pe],
                                rhs=XT[:, m * nt + b0:m * nt + b0 + bl],
                                start=(m == 0), stop=(m == k_pe - 1))
                        nc.scalar.activation(
                            STb[:, b0:b0 + bl], ssc[:, :bl], ACTF.Copy)
                    # S^T[:, (p ct + c)] view for per-chunk re-blocking
                    STv = STb[:].rearrange("q (p c) -> q c p", c=ct)

                out_sb = opool.tile([128, nt], BF16, tag="osb")

                for g0 in range(0, ct, GROUP):
                    gl = min(GROUP, ct - g0)
                    mc = md * gl
                    ps = psum.tile([128, GROUP * D], F32, tag="ps")

                    # re-block PE scores to node-major via tiny identity
                    # matmuls: out[p, m] = S^T[m, p*ct + c]
                    if k_pe:
                        ps_sc = pscp.tile([128, GROUP * 16], F32, tag="psc")
                        for cg in range(gl):
                            nc.tensor.matmul(
                                out=ps_sc[:, cg * 16:(cg + 1) * 16],
                                lhsT=STv[:, g0 + cg, :],
                                rhs=icat[0:k_pe, 0:16],
                                start=True, stop=True)

                    # DVE score path for m >= k_pe
                    P = ppool.tile([128, NMETA * GROUP * D], BF16, tag="P")
                    Pv = P[:, :md * GROUP * D].rearrange(
                        "p (m c d) -> p m c d", m=md, c=GROUP)
                    nc.vector.tensor_tensor(
                        out=Pv[:, :, :gl, :],
                        in0=Xv[:, k_pe:, g0:g0 + gl, :],
                        in1=wb[:].rearrange("p (m d) -> p m d", m=NMETA)
                              [:, k_pe:, :]
                              .unsqueeze(2).broadcast_to([128, md, gl, D]),
                        op=ALU.mult,
                    )
                    # scores[p, (m c)] = sum_d P: fold d 128->16 in bf16 (2x
                    # DVE), then one 1x tensor_reduce.
                    Pg = Pv[:, :, :gl, :]
                    f1 = fpool.tile([128, NMETA * GROUP * D // 2], BF16, tag="f1")
                    f1v = f1[:, :md * GROUP * D // 2].rearrange(
                        "p (m c d) -> p m c d", m=md, c=GROUP)[:, :, :gl, :]
                    nc.vector.tensor_tensor(
                        out=f1v, in0=Pg[:, :, :, 0:64], in1=Pg[:, :, :, 64:128],
                        op=ALU.add)
                    f2 = fpool.tile([128, NMETA * GROUP * D // 4], BF16, tag="f2")
                    f2v = f2[:, :md * GROUP * D // 4].rearrange(
                        "p (m c d) -> p m c d", m=md, c=GROUP)[:, :, :gl, :]
                    nc.vector.tensor_tensor(
                        out=f2v, in0=f1v[:, :, :, 0:32], in1=f1v[:, :, :, 32:64],
                        op=ALU.add)
                    f3 = fpool.tile([128, NMETA * GROUP * D // 8], BF16, tag="f3")
                    f3v = f3[:, :md * GROUP * D // 8].rearrange(
                        "p (m c d) -> p m c d", m=md, c=GROUP)[:, :, :gl, :]
                    nc.vector.tensor_tensor(
                        out=f3v, in0=f2v[:, :, :, 0:16], in1=f2v[:, :, :, 16:32],
                        op=ALU.add)
                    scores = spool.tile([128, GROUP * NMETA], F32, tag="scores")
                    nc.vector.tensor_reduce(
                        out=scores[:, :mc].rearrange("p (m c) -> p m c", m=md),
                        in_=f3v,
                        axis=mybir.AxisListType.X, op=ALU.add)

                    # softmax over m: merge both score pieces into one
                    # (c m)-major e tile: e = exp(relu(s)) = max(exp(s), 1)
                    e_bf = spool.tile([128, GROUP * NMETA], BF16, tag="ebf")
                    e_cm = e_bf[:, :gl * NMETA].rearrange(
                        "p (c m) -> p c m", m=NMETA)
                    if k_pe:
                        nc.scalar.activation(
                            e_cm[:, :, 0:k_pe],
                            ps_sc[:, :gl * 16].rearrange(
                                "p (c s) -> p c s", s=16)[:, :, 0:k_pe],
                            ACTF.Exp)
                    nc.scalar.activation(
                        e_cm[:, :, k_pe:].rearrange("p c m -> p m c"),
                        scores[:, :mc].rearrange("p (m c) -> p m c", m=md),
                        ACTF.Exp)
                    e_max = spool.tile([128, GROUP * NMETA], BF16, tag="emax")
                    nc.vector.tensor_scalar(
                        e_max[:, :gl * NMETA], e_bf[:, :gl * NMETA],
                        1.0, None, ALU.max)
                    em_cm = e_max[:, :gl * NMETA].rearrange(
                        "p (c m) -> p c m", m=NMETA)
                    sums = spool.tile([128, GROUP], F32, tag="sums")
                    nc.vector.tensor_reduce(
                        out=sums[:, :gl], in_=em_cm,
                        axis=mybir.AxisListType.X,
                        op=ALU.add,
                    )
                    inv = spool.tile([128, GROUP], F32, tag="inv")
                    nc.vector.reciprocal(inv[:, :gl], sums[:, :gl])

                    for cg in range(gl):
                        c = g0 + cg
                        diag = dpool.tile([128, NMETA * D], BF16, tag="diag")
                        use_dve = (diag_dve_every and
                                   chunk_idx % diag_dve_every == 0)
                        if use_dve:
                            att_f = spool.tile([128, NMETA], F32, tag="attf")
                            nc.vector.tensor_scalar(
                                att_f[:], em_cm[:, cg, :],
                                inv[:, cg:cg + 1], None, ALU.mult)
                            for m in range(NMETA):
                                nc.vector.tensor_scalar(
                                    diag[:, m * D:(m + 1) * D],
                                    icat[:, m * D:(m + 1) * D],
                                    att_f[:, m:m + 1], None, ALU.mult)
                        else:
                            att = spool.tile([128, NMETA], BF16, tag="att")
                            nc.vector.tensor_scalar(
                                att[:], em_cm[:, cg, :],
                                inv[:, cg:cg + 1], None, ALU.mult)
                            nc.gpsimd.local_scatter(
                                diag[:], att[:], sidx[:],
                                channels=128, num_elems=NMETA * D,
                                num_idxs=NMETA)
                        for m in range(NMETA):
                            nc.tensor.matmul(
                                out=ps[:, cg * D:(cg + 1) * D],
                                lhsT=diag[:, m * D:(m + 1) * D],
                                rhs=Xv[:, m, c, :],
                                start=(m == 0),
                                stop=(m == NMETA - 1),
                            )
                        chunk_idx += 1

                    # elu(x) = max(x, exp(min(x,0)) - 1)
                    w = gl * D
                    t = epool.tile([128, GROUP * D], F32, tag="t")
                    nc.scalar.activation(t[:, :w], ps[:, :w], ACTF.Relu,
                                         scale=-1.0)
                    e2 = epool.tile([128, GROUP * D], F32, tag="e2")
                    nc.scalar.activation(e2[:, :w], t[:, :w], ACTF.Exp,
                                         scale=-1.0)
                    # out = max(ps, e2 - 1) in one fused op
                    eng = nc.gpsimd if comb_on_pool else nc.vector
                    eng.scalar_tensor_tensor(
                        out=out_sb[:, g0 * D:g0 * D + w],
                        in0=e2[:, :w], scalar=-1.0, in1=ps[:, :w],
                        op0=ALU.add, op1=ALU.max)

                dsto = out_d[n0:n0 + nt, :].rearrange("(p c) d -> p (c d)", p=128)
                nc.sync.dma_start(dsto, out_sb[:])
                n0 += nt


def host_inputs(x_np, w_np, nc_pad=NC_PAD):
    """Build per-core input maps from full fp32 inputs."""
    in_maps = []
    wbig = np.ascontiguousarray(
        np.broadcast_to(w_np.T.reshape(1, NMETA * D), (128, NMETA * D))
    ).astype(ml_dtypes.bfloat16)
    sidx = (np.arange(NMETA)[None, :] * D + np.arange(128)[:, None]).astype(np.int16)
    icat = np.ascontiguousarray(
        np.tile(np.eye(128, dtype=np.float32), (1, NMETA))
    ).astype(ml_dtypes.bfloat16)
    # masked-W blocks: wmk[:, m*NMETA + m'] = W[:, m] iff m' == m else 0
    wmk_f = np.zeros((128, NMETA * NMETA), dtype=np.float32)
    for m in range(NMETA):
        wmk_f[:, m * NMETA + m] = w_np[:, m]
    wmk = wmk_f.astype(ml_dtypes.bfloat16)
    nc_raw = x_np.shape[1] // NCORES
    for c in range(NCORES):
        xs = x_np[:, c * nc_raw:(c + 1) * nc_raw, :]
        xp = np.zeros((NMETA, nc_pad, D), dtype=ml_dtypes.bfloat16)
        xp[:, :nc_raw, :] = xs.astype(ml_dtypes.bfloat16)
        blocks = []
        n0 = 0
        while n0 < nc_pad:
            nt = min(T_CHUNKS * CHUNK, nc_pad - n0)
            blocks.append(
                np.ascontiguousarray(xp[:K_PE, n0:n0 + nt, :])
                .reshape(K_PE * nt, D))
            n0 += nt
        xtl = np.concatenate(blocks, axis=0)
        in_maps.append({"x": xp, "wb": wbig, "sidx": sidx, "icat": icat,
                        "wm": wmk, "xt": xtl})
    return in_maps


_CACHE = {}


def build(reps=1, **kw):
    key = (reps, tuple(sorted(kw.items())))
    if key in _CACHE:
        return _CACHE[key]
    nc = bacc.Bacc("TRN2", target_bir_lowering=False, debug=False,
                   num_devices=NCORES)
    x = nc.dram_tensor("x", [NMETA, NC_PAD, D], BF16, kind="ExternalInput").ap()
    wb = nc.dram_tensor("wb", [128, NMETA * D], BF16, kind="ExternalInput").ap()
    sidx = nc.dram_tensor("sidx", [128, NMETA], I16, kind="ExternalInput").ap()
    icat = nc.dram_tensor("icat", [128, NMETA * D], BF16, kind="ExternalInput").ap()
    wm = nc.dram_tensor("wm", [128, NMETA * NMETA], BF16,
                        kind="ExternalInput").ap()
    xt = nc.dram_tensor("xt", [K_PE * NC_PAD, D], BF16,
                        kind="ExternalInput").ap()
    out = nc.dram_tensor("out", [NC_PAD, D], BF16, kind="ExternalOutput").ap()
    with tile.TileContext(nc) as tc:
        kernel_body(tc, out, x, wb, sidx, icat, wm, xt, reps=reps, **kw)
    nc.compile()
    _CACHE[key] = nc
    return nc


def run(input, W, trace=False, **trace_kwargs):
    x_np = np.asarray(input, dtype=np.float32)
    w_np = np.asarray(W, dtype=np.float32)
    nc = build()
    in_maps = host_inputs(x_np, w_np)
    res = bass_utils.run_bass_kernel_spmd(
        nc, in_maps, core_ids=list(range(NCORES)), trace=trace, **trace_kwargs)
    nc_raw = x_np.shape[1] // NCORES
    full = np.concatenate(
        [np.asarray(res.results[c]["out"][:nc_raw], dtype=np.float32)
         for c in range(NCORES)], axis=0)
    return full, res


def kernel(input, W):
    out, _ = run(input, W, trace=False)
    return out


# ---------------------------------------------------------------------------
# Timing harness (test-only): persistent jit over the bass_exec primitive so
# repeated executions reuse device-resident inputs. HW kernel time is derived
# from the slope between an R-repeat NEFF and the 1-repeat NEFF.
# ---------------------------------------------------------------------------

def make_runner(nc):
    import jax
    from jax.experimental.shard_map import shard_map
    from jax.sharding import Mesh, PartitionSpec, NamedSharding
    from concourse import bass2jax as b2j

    b2j.install_neuronx_cc_hook()
    partition_name = nc.partition_id_tensor.name if nc.partition_id_tensor else None
    in_names, out_names, out_avals, zero_outs = [], [], [], []
    for alloc in nc.m.functions[0].allocations:
        if not isinstance(alloc, mybir.MemoryLocationSet):
            continue
        name = alloc.memorylocations[0].name
        if alloc.kind == "ExternalInput":
            if name != partition_name:
                in_names.append(name)
        elif alloc.kind == "ExternalOutput":
            out_names.append(name)
            shape = tuple(alloc.tensor_shape)
            dtype = mybir.dt.np(alloc.dtype)
            out_avals.append(jax.core.ShapedArray(shape, dtype))
            zero_outs.append(np.zeros(shape, dtype))
    n_params = len(in_names)
    n_outs = len(out_avals)
    all_names = in_names + out_names + ([partition_name] if partition_name else [])

    def _body(*args):
        operands = list(args)
        if partition_name is not None:
            operands.append(b2j.partition_id_tensor())
        outs = b2j._bass_exec_p.bind(
            *operands,
            out_avals=tuple(out_avals),
            in_names=tuple(all_names),
            out_names=tuple(out_names),
            lowering_input_output_aliases=(),
            sim_require_finite=True,
            sim_require_nnan=True,
            nc=nc,
        )
        return tuple(outs)

    devices = jax.devices()[:NCORES]
    mesh = Mesh(np.asarray(devices), ("core",))
    in_specs = (PartitionSpec("core"),) * (n_params + n_outs)
    out_specs = (PartitionSpec("core"),) * n_outs
    donate = tuple(range(n_params, n_params + n_outs))
    sharded = jax.jit(
        shard_map(_body, mesh=mesh, in_specs=in_specs, out_specs=out_specs,
                  check_rep=False),
        donate_argnums=donate, keep_unused=True)
    sharding = NamedSharding(mesh, PartitionSpec("core"))
    return sharded, in_names, zero_outs, sharding


class _TimedRunner:
    def __init__(self, nc, in_maps):
        import jax
        self.jax = jax
        sharded, in_names, zero_outs, sharding = make_runner(nc)
        self.sharded = sharded
        concat_in = [
            np.concatenate([in_maps[c][n] for c in range(NCORES)], axis=0)
            for n in in_names
        ]
        self.xs = [jax.device_put(a, sharding) for a in concat_in]
        self.zero_outs = zero_outs
        self.sharding = sharding

    def _zset(self):
        return [
            self.jax.device_put(
                np.zeros((NCORES * z.shape[0], *z.shape[1:]), z.dtype),
                self.sharding)
            for z in self.zero_outs
        ]

    def piped(self, reps):
        import time as _t
        zsets = [self._zset() for _ in range(reps + 1)]
        self.jax.block_until_ready(zsets)
        self.jax.block_until_ready(self.xs)
        o = self.sharded(*self.xs, *zsets[0])
        self.jax.block_until_ready(o)
        _ = self.jax.device_get(o[0])
        t0 = _t.perf_counter()
        outs = [self.sharded(*self.xs, *zsets[1 + k]) for k in range(reps)]
        self.jax.block_until_ready(outs)
        # force true device completion: fetch the last output's bytes
        _ = self.jax.device_get(outs[-1][0])
        return (_t.perf_counter() - t0) / reps


def measure(input, W, reps=12, neff_reps=9, rounds=4, **kw):
    """Estimate per-iteration HW time via multi-repeat NEFF slope.

    Interleaves rounds of (1-repeat NEFF, R-repeat NEFF) piped timings and
    takes the min across rounds for each to reject dispatch-overhead noise.
    """
    x_np = np.asarray(input, dtype=np.float32)
    w_np = np.asarray(W, dtype=np.float32)
    in_maps = host_inputs(x_np, w_np)

    nc1 = build(reps=1, **kw)
    ncr = build(reps=neff_reps, **kw)
    r1 = _TimedRunner(nc1, in_maps)
    rr = _TimedRunner(ncr, in_maps)
    t1s, trs = [], []
    for _ in range(rounds):
        t1s.append(r1.piped(reps))
        trs.append(rr.piped(reps))
    t1, tr = min(t1s), min(trs)
    slope = (tr - t1) / (neff_reps - 1)
    return t1, tr, slope, t1s, trs

